# revision 1
# baseline (speedup 1.0000x reference)
"""Trainium2 Bass kernel for nn_AttentionAugmentation.

Attention with 2D relative-position logits. B=8, H=W=32, dk=dv=256, Nh=8.
Sharding: data-parallel over batch (one batch per NeuronCore, 8 cores).

Per-core algorithm (one batch, 8 heads of 1024x1024 attention, dkh=32):
  - load x [1024, 768] into SBUF bf16, pixel-major.
  - PE-transpose q/k in 4-head groups [128, 128] -> psum [128 ch, 1024 pix],
    DVE copy to a scratch (scaling q by dkh^-0.5), then SBUF->SBUF DMAs
    scatter each head's 32 rows to partitions 0-31 of the augmented tensors.
  - relative logits folded into the main QK matmul via a 96-row augmented
    contraction: rows 0-31 qT/kT, rows 32-63 (WRELT | one-hot of key y2),
    rows 64-95 (HRELT | one-hot of key x2). WRELT/HRELT computed with one
    small matmul per (y, direction) over all heads using slices of
    key_rel^T (implements the rel->abs skew exactly).
  - S^T = kaug^T @ qaug per 128-key chunk (keys on partitions, queries on
    the free axis), exp on ScalarE in [128, 2048] chunks (no max
    subtraction: logits are O(10) for randn inputs), AV with lhsT=[V | 1]
    giving [33, 1024] = attn^T rows + softmax denominators in one pass.
  - PE-transpose attn^T back to [128, 33] chunks, scale by reciprocal
    denominators (per-partition scalar), assemble [1024, 256] output.
Matmul operands are bf16 (PSUM accumulation in fp32).
"""
import sys

sys.path.insert(0, "/opt/trn_rl_repo")

from contextlib import ExitStack

import numpy as np

import concourse.bass as bass
from concourse import bacc
import concourse.mybir as mybir
from concourse import masks
from concourse.tile import TileContext

HW = 1024
CH = 768
NH = 8
F32 = mybir.dt.float32
BF16 = mybir.dt.bfloat16
EXP = mybir.ActivationFunctionType.Exp
QSCALE = float((256 / 8) ** -0.5)


def build_nc():
    nc = bacc.Bacc()
    # input split in two halves: a single [1024, 768] parameter makes the
    # axon-pjrt reshard program's dynamic-slice exceed a 16-bit semaphore
    # field in neuronx-cc (25MB concat across 8 cores), crashing walrus.
    xa_d = nc.declare_dram_parameter("xa", [HW // 2, CH], F32, isOutput=False)
    xb_d = nc.declare_dram_parameter("xb", [HW // 2, CH], F32, isOutput=False)
    krw_d = nc.declare_dram_parameter("krw", [63, 32], F32, isOutput=False)
    krh_d = nc.declare_dram_parameter("krh", [63, 32], F32, isOutput=False)
    out_d = nc.declare_dram_parameter("out", [HW, 256], F32, isOutput=True)

    with ExitStack() as octx:
        tc = octx.enter_context(TileContext(nc))
        sb = octx.enter_context(tc.tile_pool(name="persist", bufs=1))

        x_sb = sb.tile([128, 8 * CH], BF16)     # natural input: part p, col 768c+ch
        qaug = sb.tile([96, NH * HW], BF16)     # per head h: cols 1024h + (32x + y)
        kaug = sb.tile([96, NH * HW], BF16)
        v1 = sb.tile([128, NH * 8 * 33], BF16)  # per (h,c): 33 cols = V chunk | ones
        tscr = sb.tile([128, 4 * HW], BF16)     # transpose scratch: (kind, group)
        out_sb = sb.tile([128, 8 * 256], F32)   # col 256c + ch
        identb = sb.tile([128, 128], BF16)
        identf = sb.tile([128, 128], F32)
        krw_sb = sb.tile([63, 32], BF16)
        krh_sb = sb.tile([63, 32], BF16)
        krwT = sb.tile([32, 63], BF16)
        krhT = sb.tile([32, 63], BF16)

        # ---- input DMAs (SWDGE: fp32 -> bf16 cast), per-chunk for overlap ----
        for c in range(8):
            src_d = xa_d if c < 4 else xb_d
            cc = c % 4
            nc.gpsimd.dma_start(
                out=x_sb[:, 768 * c:768 * c + 768],
                in_=src_d[128 * cc:128 * cc + 128, :],
            )
        nc.gpsimd.dma_start(out=krw_sb[:], in_=krw_d[:])
        nc.gpsimd.dma_start(out=krh_sb[:], in_=krh_d[:])

        # ---- constants ----
        masks.make_identity(nc, identb[:])
        masks.make_identity(nc, identf[:])
        # one-hot rows of kaug, head-0 block only: rows 32-63: [y2(k)==j],
        # rows 64-95: [x2(k)==j]; col = 32*x2 + y2. Then DMA-replicate to
        # the other 7 head blocks (log-doubling).
        nc.gpsimd.memset(kaug[32:64, 0:HW], 0.0)
        nc.gpsimd.memset(kaug[64:96, 0:HW], 0.0)
        nc.gpsimd.affine_select(
            out=kaug[32:64, 0:HW].rearrange("p (x y) -> p x y", x=32, y=32),
            in_=kaug[32:64, 0:HW].rearrange("p (x y) -> p x y", x=32, y=32),
            compare_op=mybir.AluOpType.not_equal,
            fill=1.0,
            base=0,
            pattern=[[0, 32], [-1, 32]],
            channel_multiplier=1,
        )
        nc.gpsimd.affine_select(
            out=kaug[64:96, 0:HW].rearrange("p (x y) -> p x y", x=32, y=32),
            in_=kaug[64:96, 0:HW].rearrange("p (x y) -> p x y", x=32, y=32),
            compare_op=mybir.AluOpType.not_equal,
            fill=1.0,
            base=0,
            pattern=[[-1, 32], [0, 32]],
            channel_multiplier=1,
        )
        for base in (32, 64):
            n = HW
            while n < NH * HW:
                rep = min(n, NH * HW - n)
                nc.sync.dma_start(
                    out=kaug[base:base + 32, n:n + rep],
                    in_=kaug[base:base + 32, 0:rep],
                )
                n += rep
        # V1: ones everywhere, then V chunks overwrite cols 0-31 of each 33-block
        nc.vector.memset(v1[:], 1.0)

        # ================= Phase A: transposes + rel logits =================
        with ExitStack() as actx:
            psA = actx.enter_context(tc.tile_pool(name="psA", bufs=2, space="PSUM"))
            psR = actx.enter_context(tc.tile_pool(name="psR", bufs=1, space="PSUM"))
            psW = actx.enter_context(tc.tile_pool(name="psW", bufs=2, space="PSUM"))

            # PE warm-up: ~6us of back-to-back matmuls so the HAM clock gate
            # opens (1.2 -> 2.4 GHz) before the real work; later gaps stay
            # under the ~3.4us re-throttle window.
            wps = psA.tile([128, HW], BF16, tag="tps")
            for _ in range(18):
                nc.tensor.transpose(
                    out=wps[:, 0:128], in_=identb[:], identity=identb[:],
                )

            # key_rel transposes: krw [63,32] -> krwT [32,63]
            for srct, dst in ((krw_sb, krwT), (krh_sb, krhT)):
                pt = psA.tile([128, HW], BF16, tag="tps")
                nc.tensor.transpose(
                    out=pt[0:32, 0:63], in_=srct[:], identity=identb[0:63, 0:63]
                )
                nc.vector.tensor_copy(out=dst[:], in_=pt[0:32, 0:63])

            # qT / kT: PE-transpose 4-head groups, DVE copy (scale q), then
            # partition-scatter DMAs into qaug/kaug rows 0-31.
            for kind in range(2):       # 0: q, 1: k
                for g in range(2):      # head group: heads 4g..4g+3
                    col0 = 256 * kind + 128 * g
                    pt = psA.tile([128, HW], BF16, tag="tps")
                    for c in range(8):
                        nc.tensor.transpose(
                            out=pt[:, 128 * c:128 * c + 128],
                            in_=x_sb[:, 768 * c + col0:768 * c + col0 + 128],
                            identity=identb[:, 0:128],
                        )
                    scr = tscr[:, HW * (2 * kind + g):HW * (2 * kind + g) + HW]
                    if kind == 0:
                        nc.vector.tensor_scalar_mul(out=scr, in0=pt[:], scalar1=QSCALE)
                    else:
                        nc.vector.tensor_copy(out=scr, in_=pt[:])
                    dstt = qaug if kind == 0 else kaug
                    for hh in range(4):
                        h = 4 * g + hh
                        nc.sync.dma_start(
                            out=dstt[0:32, HW * h:HW * h + HW],
                            in_=tscr[32 * hh:32 * hh + 32,
                                     HW * (2 * kind + g):HW * (2 * kind + g) + HW],
                        )

            # V chunks (cols 0-31 of each 33-block), one copy per chunk
            v1v = v1[:].rearrange("p (h c e) -> p c h e", h=8, c=8, e=33)
            for c in range(8):
                nc.vector.tensor_copy(
                    out=v1v[:, c, :, 0:32],
                    in_=x_sb[:, 768 * c + 512:768 * c + 512 + 256]
                        .rearrange("p (h e) -> p h e", h=8),
                )

            # rel logits.
            # H direction: per head-pair accumulate 32 x-positions into one
            # [32, 2048] psum laid out (x, h, y) -- each matmul writes a
            # contiguous 64-col block (bank-safe) -- then one strided-run
            # copy into qaug rows 64-95.
            qa4 = qaug[0:32, :].rearrange("p (h x y) -> p h x y", h=8, x=32, y=32)
            hdst = qaug[64:96, :].rearrange("p (h x y) -> p h x y", h=8, x=32, y=32)
            for pr in range(4):  # heads 2*pr, 2*pr+1
                pp = psR.tile([32, 2048], F32, tag="relps")
                pv = pp[:].rearrange("p (x h y) -> p x h y", x=32, h=2, y=32)
                for i in range(32):
                    nc.tensor.matmul(
                        out=pv[:, i, :, :],
                        lhsT=krhT[:, 31 - i:63 - i],
                        rhs=qa4[:, 2 * pr:2 * pr + 2, i, :],
                        start=True, stop=True,
                    )
                dst = hdst[:, 2 * pr:2 * pr + 2, :, :]
                eng = nc.vector if pr % 2 == 0 else nc.scalar
                if pr % 2 == 0:
                    nc.vector.tensor_copy(
                        out=dst, in_=pv[:].rearrange("p x h y -> p h x y"))
                else:
                    nc.scalar.copy(
                        out=dst, in_=pv[:].rearrange("p x h y -> p h x y"))
            # W direction: per y-position one matmul over all heads; the dst
            # runs are single elements (skew), split copies across DVE/ACT.
            wdst = qaug[32:64, :].rearrange("p (h x y) -> p h x y", h=8, x=32, y=32)
            for i in range(32):
                pw = psW.tile([32, 256], F32, tag="relw")
                nc.tensor.matmul(
                    out=pw[:],
                    lhsT=krwT[:, 31 - i:63 - i],
                    rhs=qa4[:, :, :, i],
                    start=True, stop=True,
                )
                if i % 2 == 0:
                    nc.vector.tensor_copy(
                        out=wdst[:, :, :, i],
                        in_=pw[:].rearrange("p (h x) -> p h x", h=8),
                    )
                else:
                    nc.scalar.copy(
                        out=wdst[:, :, :, i],
                        in_=pw[:].rearrange("p (h x) -> p h x", h=8),
                    )

        # ================= Phase B: attention per head =================
        with ExitStack() as bctx:
            psS = bctx.enter_context(tc.tile_pool(name="psS", bufs=2, space="PSUM"))
            psT = bctx.enter_context(tc.tile_pool(name="psT", bufs=1, space="PSUM"))
            psO = bctx.enter_context(tc.tile_pool(name="psO", bufs=2, space="PSUM"))
            sbW = bctx.enter_context(tc.tile_pool(name="sbW", bufs=3))
            sbA = bctx.enter_context(tc.tile_pool(name="sbA", bufs=2))
            sbR = bctx.enter_context(tc.tile_pool(name="sbR", bufs=3))

            for hp in range(NH // 2):
                # two heads share one att psum: head 2hp at partitions 0-32,
                # head 2hp+1 at partitions 64-96 (col-group aligned).
                att = psT.tile([97, HW], F32, tag="att")
                for hh in range(2):
                    h = 2 * hp + hh
                    pb = 64 * hh
                    for c in range(8):
                        s_ps = psS.tile([128, HW], F32, tag="sT")
                        for e in range(2):
                            nc.tensor.matmul(
                                out=s_ps[:, 512 * e:512 * e + 512],
                                lhsT=kaug[:, HW * h + 128 * c:HW * h + 128 * c + 128],
                                rhs=qaug[:, HW * h + 512 * e:HW * h + 512 * e + 512],
                                start=True, stop=True,
                            )
                        wexp = sbW.tile([128, HW], BF16, tag="wexp")
                        nc.scalar.activation(out=wexp[:], in_=s_ps[:], func=EXP)
                        for e in range(2):
                            nc.tensor.matmul(
                                out=att[pb:pb + 33, 512 * e:512 * e + 512],
                                lhsT=v1[:, 264 * h + 33 * c:264 * h + 33 * c + 33],
                                rhs=wexp[:, 512 * e:512 * e + 512],
                                start=(c == 0), stop=(c == 7),
                            )
                att_sb = sbA.tile([97, HW], F32, tag="attsb")
                nc.scalar.copy(out=att_sb[:], in_=att[:])
                for c in range(8):
                    ot = psO.tile([128, 97], F32, tag="ot")
                    nc.tensor.transpose(
                        out=ot[:],
                        in_=att_sb[:, 128 * c:128 * c + 128],
                        identity=identf[0:97, 0:97],
                    )
                    rc = sbR.tile([128, 2], F32, tag="rc")
                    nc.vector.reciprocal(out=rc[:], in_=ot[:, 32:97:64])
                    for hh in range(2):
                        nc.vector.tensor_scalar_mul(
                            out=out_sb[:, 256 * c + 32 * (2 * hp + hh):
                                       256 * c + 32 * (2 * hp + hh) + 32],
                            in0=ot[:, 64 * hh:64 * hh + 32],
                            scalar1=rc[:, hh:hh + 1],
                        )

        # ---- output DMA ----
        nc.sync.dma_start(
            out=out_d[:].rearrange("(c p) d -> p c d", p=128),
            in_=out_sb[:].rearrange("p (c d) -> p c d", c=8),
        )
    if not nc.is_finalized():
        nc.finalize()
    return nc


_NC = None


def _ensure_axon_hooks_module():
    """bass_utils imports antenv.axon_hooks unconditionally when trace=True;
    this image's antenv lacks it. Provide a stub so tracing degrades to
    no-trace instead of crashing (a real hook can be set by a profiler)."""
    import types

    if "antenv.axon_hooks" in sys.modules:
        return
    try:
        import antenv.axon_hooks  # noqa: F401
        return
    except ImportError:
        pass
    try:
        import antenv
    except ImportError:
        return
    m = types.ModuleType("antenv.axon_hooks")
    m._hook = None
    m.get_axon_ntff_profile_hook = lambda: m._hook
    m.set_axon_ntff_profile_hook = lambda h: setattr(m, "_hook", h)
    sys.modules["antenv.axon_hooks"] = m
    antenv.axon_hooks = m


def kernel(**inputs):
    global _NC
    x = np.ascontiguousarray(np.asarray(inputs["inputs"], dtype=np.float32))
    krw = np.ascontiguousarray(np.asarray(inputs["key_rel_w"], dtype=np.float32))
    krh = np.ascontiguousarray(np.asarray(inputs["key_rel_h"], dtype=np.float32))
    assert x.shape == (8, 32, 32, 768), x.shape
    assert int(inputs["dk"]) == 256 and int(inputs["dv"]) == 256
    assert int(inputs["Nh"]) == 8

    if _NC is None:
        _NC = build_nc()
    _ensure_axon_hooks_module()
    from concourse.bass_utils import run_bass_kernel_spmd

    in_maps = [
        {
            "xa": x[b].reshape(HW, CH)[:HW // 2],
            "xb": x[b].reshape(HW, CH)[HW // 2:],
            "krw": krw,
            "krh": krh,
        }
        for b in range(8)
    ]
    res = run_bass_kernel_spmd(_NC, in_maps, list(range(8)))
    kernel.last_result = res
    out = np.stack([res.results[b]["out"].reshape(32, 32, 256) for b in range(8)], 0)
    return out


if __name__ == "__main__":
    nc = build_nc()
    print("built ok")



# revision 3
# speedup vs baseline: 1.1266x; 1.1266x over previous
"""Trainium2 Bass kernel for nn_AttentionAugmentation (v2).

Attention with 2D relative-position logits. B=8, H=W=32, dk=dv=256, Nh=8.
Sharding: data-parallel over batch (one batch per NeuronCore, 8 cores).

Per-core algorithm (one batch, 8 heads of 1024x1024 attention, dkh=32):
  - inputs loaded as three column-split casting DMAs (q, k, v) so the q
    transposes / rel-logit matmuls start ~3us in, before k/v land.
  - PE warm-up with real matmuls (transposes don't open the HAM clock gate).
  - q/k PE-transposed in 4-head groups -> DVE copy (q scaled) -> partition-
    scatter DMAs into rows 0-31 of the augmented operands qaug/kaug.
  - relative logits folded into the main QK matmul via a 96-row augmented
    contraction: rows 32-63 (WRELT | one-hot of key y2), rows 64-95
    (HRELT | one-hot of key x2). WRELT/HRELT built with one small matmul
    per (y or x, 4-batch) over all heads; psum batched [32,1024] and copied
    with wide 4D-AP copies split across ACT/DVE (the old per-slice scalar
    copies serialized the whole front half of the kernel).
  - S^T = kaug^T @ qaug per 128-key chunk; exp split between ScalarE
    (true exp) and VectorE (Schraudolph: (int16)(A*S + B) bitcast as bf16
    approximates e^S to ~3%; C tuned for zero mean bias so ACT- and
    DVE-exp'd key chunks agree); AV with lhsT=[V | 1] giving attn^T rows
    plus softmax denominators.
  - attn^T PE-transposed back per 128-query chunk, scaled by reciprocal
    denominators into out_sb; per-pair output DMAs overlap the tail.
Matmul operands bf16 (fp32 PSUM accumulation).
"""
import sys

sys.path.insert(0, "/opt/trn_rl_repo")

from contextlib import ExitStack

import numpy as np

import concourse.bass as bass
from concourse import bacc
import concourse.mybir as mybir
from concourse import masks
from concourse.tile import TileContext

HW = 1024
CH = 768
NH = 8
F32 = mybir.dt.float32
BF16 = mybir.dt.bfloat16
I16 = mybir.dt.int16
EXP = mybir.ActivationFunctionType.Exp
MULT = mybir.AluOpType.mult
ADD = mybir.AluOpType.add
QSCALE = float((256 / 8) ** -0.5)
# Schraudolph exp for bf16: (int16)(A*x + B) bits ~= bf16(e^x).
SCH_A = 184.6649652337873      # 2^7 / ln(2)
SCH_B = 16256.0 - 7.0          # 127 * 2^7 + C (C=-7: zero mean ratio bias)
DVE_CHUNKS = (2, 5, 7)         # key chunks exp'd on VectorE (rest ScalarE)


def build_nc():
    nc = bacc.Bacc()
    # input split in two halves: a single [1024, 768] parameter makes the
    # axon-pjrt reshard program's dynamic-slice exceed a 16-bit semaphore
    # field in neuronx-cc (25MB concat across 8 cores), crashing walrus.
    xa_d = nc.declare_dram_parameter("xa", [HW // 2, CH], F32, isOutput=False)
    xb_d = nc.declare_dram_parameter("xb", [HW // 2, CH], F32, isOutput=False)
    krw_d = nc.declare_dram_parameter("krw", [63, 32], F32, isOutput=False)
    krh_d = nc.declare_dram_parameter("krh", [63, 32], F32, isOutput=False)
    out_d = nc.declare_dram_parameter("out", [HW, 256], F32, isOutput=True)

    with ExitStack() as octx:
        tc = octx.enter_context(TileContext(nc))
        sb = octx.enter_context(tc.tile_pool(name="persist", bufs=1))

        q_sb = sb.tile([128, 2048], BF16)       # (c, e): chunk c, q-channel e
        k_sb = sb.tile([128, 2048], BF16)
        v_sb = sb.tile([128, 2048], BF16)
        qaug = sb.tile([96, NH * HW], BF16)     # per head h: cols 1024h + (32x + y)
        kaug = sb.tile([96, NH * HW], BF16)
        v1 = sb.tile([128, NH * 8 * 33], BF16)  # per (h,c): 33 cols = V chunk | ones
        tscr = sb.tile([128, 4 * HW], BF16)     # transpose scratch (kind, group)
        out_sb = sb.tile([128, 8 * 256], F32)   # col 256c + 32h + d
        identb = sb.tile([128, 128], BF16)
        identf = sb.tile([128, 128], F32)
        krw_sb = sb.tile([63, 32], BF16)
        krh_sb = sb.tile([63, 32], BF16)
        krwT = sb.tile([32, 63], BF16)
        krhT = sb.tile([32, 63], BF16)

        # ---- identity first (gates PE warm-up), then input DMAs ----
        masks.make_identity(nc, identb[:])

        for kind, dst in ((0, q_sb), (1, k_sb), (2, v_sb)):
            for half, src_d in ((0, xa_d), (1, xb_d)):
                nc.gpsimd.dma_start(
                    out=dst[:].rearrange("p (c e) -> p c e", c=8)
                        [:, 4 * half:4 * half + 4, :],
                    in_=src_d[:, 256 * kind:256 * kind + 256]
                        .rearrange("(c p) e -> p c e", p=128),
                )
        nc.gpsimd.dma_start(out=krw_sb[:], in_=krw_d[:])
        nc.gpsimd.dma_start(out=krh_sb[:], in_=krh_d[:])

        # ---- remaining constants (gpsimd queue, after DMA issues) ----
        masks.make_identity(nc, identf[:])
        nc.gpsimd.memset(kaug[32:64, 0:HW], 0.0)
        nc.gpsimd.memset(kaug[64:96, 0:HW], 0.0)
        # one-hot blocks, head-0 only: rows 32-63: [y2(k)==j], rows 64-95:
        # [x2(k)==j]; key col = 32*x2 + y2. DMA-replicated to heads 1-7.
        nc.gpsimd.affine_select(
            out=kaug[32:64, 0:HW].rearrange("p (x y) -> p x y", x=32, y=32),
            in_=kaug[32:64, 0:HW].rearrange("p (x y) -> p x y", x=32, y=32),
            compare_op=mybir.AluOpType.not_equal,
            fill=1.0,
            base=0,
            pattern=[[0, 32], [-1, 32]],
            channel_multiplier=1,
        )
        nc.gpsimd.affine_select(
            out=kaug[64:96, 0:HW].rearrange("p (x y) -> p x y", x=32, y=32),
            in_=kaug[64:96, 0:HW].rearrange("p (x y) -> p x y", x=32, y=32),
            compare_op=mybir.AluOpType.not_equal,
            fill=1.0,
            base=0,
            pattern=[[-1, 32], [0, 32]],
            channel_multiplier=1,
        )
        for base in (32, 64):
            n = HW
            while n < NH * HW:
                rep = min(n, NH * HW - n)
                nc.sync.dma_start(
                    out=kaug[base:base + 32, n:n + rep],
                    in_=kaug[base:base + 32, 0:rep],
                )
                n += rep
        nc.gpsimd.memset(v1[:], 1.0)

        qa4 = qaug[0:32, :].rearrange("p (h x y) -> p h x y", h=8, x=32, y=32)
        wdst = qaug[32:64, :].rearrange("p (h x y) -> p h x y", h=8, x=32, y=32)
        hdst = qaug[64:96, :].rearrange("p (h x y) -> p h x y", h=8, x=32, y=32)

        # ================= Phase A =================
        with ExitStack() as actx:
            psA = actx.enter_context(tc.tile_pool(name="psA", bufs=2, space="PSUM"))

            # PE warm-up: ~4us of real matmuls so the HAM clock gate opens
            # (transpose-mode doesn't count as PE-busy for HAM).
            wps = psA.tile([128, 512], F32, tag="warm", bufs=1)
            for _ in range(36):
                nc.tensor.matmul(
                    out=wps[:, 0:128], lhsT=identb[:], rhs=identb[:],
                    start=True, stop=True,
                )

            # key_rel transposes: krw [63,32] -> krwT [32,63]
            for srct, dst in ((krw_sb, krwT), (krh_sb, krhT)):
                pt = psA.tile([128, HW], BF16, tag="tps")
                nc.tensor.transpose(
                    out=pt[0:32, 0:63], in_=srct[:], identity=identb[0:63, 0:63]
                )
                nc.vector.tensor_copy(out=dst[:], in_=pt[0:32, 0:63])

            # qT/kT: PE-transpose 4-head groups, DVE copy (scale q), then
            # partition-scatter DMAs into qaug/kaug rows 0-31.
            def transpose_kind(kind):
                src_sb = q_sb if kind == 0 else k_sb
                dstt = qaug if kind == 0 else kaug
                for g in range(2):
                    pt = psA.tile([128, HW], BF16, tag="tps")
                    for c in range(8):
                        nc.tensor.transpose(
                            out=pt[:, 128 * c:128 * c + 128],
                            in_=src_sb[:, 256 * c + 128 * g:256 * c + 128 * g + 128],
                            identity=identb[:, 0:128],
                        )
                    scr = tscr[:, HW * (2 * kind + g):HW * (2 * kind + g) + HW]
                    if kind == 0:
                        nc.vector.tensor_scalar_mul(out=scr, in0=pt[:], scalar1=QSCALE)
                    else:
                        nc.vector.tensor_copy(out=scr, in_=pt[:])
                    for hh in range(4):
                        h = 4 * g + hh
                        nc.sync.dma_start(
                            out=dstt[0:32, HW * h:HW * h + HW],
                            in_=tscr[32 * hh:32 * hh + 32,
                                     HW * (2 * kind + g):HW * (2 * kind + g) + HW],
                        )

            transpose_kind(0)   # q (lands first)
            transpose_kind(1)   # k

            # rel logits, batched: per 4 x-positions (H) / y-positions (W)
            # one [32, 1024] psum + one wide 4D-AP copy.
            for t in range(8):
                pR = psA.tile([32, 1024], F32, tag="rel")
                pv = pR[:].rearrange("p (i h y) -> p i h y", i=4, h=8, y=32)
                for xi in range(4):
                    x = 4 * t + xi
                    nc.tensor.matmul(
                        out=pv[:, xi, :, :],
                        lhsT=krhT[:, 31 - x:63 - x],
                        rhs=qa4[:, :, x, :],
                        start=True, stop=True,
                    )
                dst = hdst[:, :, 4 * t:4 * t + 4, :]
                src = pv[:].rearrange("p i h y -> p h i y")
                if t % 2 == 0:
                    nc.vector.tensor_copy(out=dst, in_=src)
                else:
                    nc.scalar.copy(out=dst, in_=src)
            for t in range(8):
                pR = psA.tile([32, 1024], F32, tag="rel")
                pw = pR[:].rearrange("p (i h x) -> p i h x", i=4, h=8, x=32)
                for yi in range(4):
                    y = 4 * t + yi
                    nc.tensor.matmul(
                        out=pw[:, yi, :, :],
                        lhsT=krwT[:, 31 - y:63 - y],
                        rhs=qa4[:, :, :, y],
                        start=True, stop=True,
                    )
                dst = wdst[:, :, :, 4 * t:4 * t + 4]
                src = pw[:].rearrange("p i h x -> p h x i")
                if t % 2 == 0:
                    nc.vector.tensor_copy(out=dst, in_=src)
                else:
                    nc.scalar.copy(out=dst, in_=src)

            # V chunks into v1 via SBUF->SBUF DMAs (64B runs)
            v1v = v1[:].rearrange("p (h c e) -> p c h e", h=8, c=8, e=33)
            for c in range(8):
                nc.sync.dma_start(
                    out=v1v[:, c, :, 0:32],
                    in_=v_sb[:, 256 * c:256 * c + 256]
                        .rearrange("p (h e) -> p h e", h=8),
                )

        # ================= Phase B: attention =================
        with ExitStack() as bctx:
            psB = bctx.enter_context(tc.tile_pool(name="psB", bufs=1, space="PSUM"))
            sbW = bctx.enter_context(tc.tile_pool(name="sbW", bufs=3))
            sbA = bctx.enter_context(tc.tile_pool(name="sbA", bufs=2))
            sbR = bctx.enter_context(tc.tile_pool(name="sbR", bufs=3))

            for hp in range(NH // 2):
                # two heads share the att psum halves: head 2hp at partitions
                # 0-32, head 2hp+1 at 64-96 (col-group aligned).
                att = [psB.tile([97, 512], F32, tag=f"att{e}", bufs=1,
                                name=f"att{e}")
                       for e in range(2)]
                for hh in range(2):
                    h = 2 * hp + hh
                    pb = 64 * hh
                    for c in range(8):
                        s_ps = psB.tile([128, HW], F32, tag="s", bufs=2)
                        for e in range(2):
                            nc.tensor.matmul(
                                out=s_ps[:, 512 * e:512 * e + 512],
                                lhsT=kaug[:, HW * h + 128 * c:HW * h + 128 * c + 128],
                                rhs=qaug[:, HW * h + 512 * e:HW * h + 512 * e + 512],
                                start=True, stop=True,
                            )
                        wexp = sbW.tile([128, HW], BF16, tag="wexp")
                        if c in DVE_CHUNKS:
                            nc.vector.tensor_scalar(
                                out=wexp[:].bitcast(I16),
                                in0=s_ps[:],
                                scalar1=SCH_A, scalar2=SCH_B,
                                op0=MULT, op1=ADD,
                            )
                        else:
                            nc.scalar.activation(out=wexp[:], in_=s_ps[:], func=EXP)
                        for e in range(2):
                            nc.tensor.matmul(
                                out=att[e][pb:pb + 33, :],
                                lhsT=v1[:, 264 * h + 33 * c:264 * h + 33 * c + 33],
                                rhs=wexp[:, 512 * e:512 * e + 512],
                                start=(c == 0), stop=(c == 7),
                            )
                for e in range(2):
                    att_sb = sbA.tile([97, 512], F32, tag="attsb")
                    nc.scalar.copy(out=att_sb[:], in_=att[e][:])
                    for ci in range(4):
                        c = 4 * e + ci
                        ot = psB.tile([128, 97], F32, tag="ot", bufs=2)
                        nc.tensor.transpose(
                            out=ot[:],
                            in_=att_sb[:, 128 * ci:128 * ci + 128],
                            identity=identf[0:97, 0:97],
                        )
                        rc = sbR.tile([128, 2], F32, tag="rc")
                        nc.vector.reciprocal(out=rc[:], in_=ot[:, 32:97:64])
                        for hh in range(2):
                            nc.vector.tensor_scalar_mul(
                                out=out_sb[:, 256 * c + 64 * hp + 32 * hh:
                                           256 * c + 64 * hp + 32 * hh + 32],
                                in0=ot[:, 64 * hh:64 * hh + 32],
                                scalar1=rc[:, hh:hh + 1],
                            )
                # per-pair output DMA (cols 64hp..64hp+63 of each 256-block)
                nc.sync.dma_start(
                    out=out_d[:].rearrange("(c p) d -> p c d", p=128)
                        [:, :, 64 * hp:64 * hp + 64],
                    in_=out_sb[:].rearrange("p (c d) -> p c d", c=8)
                        [:, :, 64 * hp:64 * hp + 64],
                )
    if not nc.is_finalized():
        nc.finalize()
    return nc


_NC = None


def _ensure_axon_hooks_module():
    """bass_utils imports antenv.axon_hooks unconditionally when trace=True;
    this image's antenv lacks it. Provide a stub so tracing degrades to
    no-trace instead of crashing (a real hook can be set by a profiler)."""
    import types

    if "antenv.axon_hooks" in sys.modules:
        return
    try:
        import antenv.axon_hooks  # noqa: F401
        return
    except ImportError:
        pass
    try:
        import antenv
    except ImportError:
        return
    m = types.ModuleType("antenv.axon_hooks")
    m._hook = None
    m.get_axon_ntff_profile_hook = lambda: m._hook
    m.set_axon_ntff_profile_hook = lambda h: setattr(m, "_hook", h)
    sys.modules["antenv.axon_hooks"] = m
    antenv.axon_hooks = m


def kernel(**inputs):
    global _NC
    x = np.ascontiguousarray(np.asarray(inputs["inputs"], dtype=np.float32))
    krw = np.ascontiguousarray(np.asarray(inputs["key_rel_w"], dtype=np.float32))
    krh = np.ascontiguousarray(np.asarray(inputs["key_rel_h"], dtype=np.float32))
    assert x.shape == (8, 32, 32, 768), x.shape
    assert int(inputs["dk"]) == 256 and int(inputs["dv"]) == 256
    assert int(inputs["Nh"]) == 8

    if _NC is None:
        _NC = build_nc()
    _ensure_axon_hooks_module()
    from concourse.bass_utils import run_bass_kernel_spmd

    in_maps = [
        {
            "xa": x[b].reshape(HW, CH)[:HW // 2],
            "xb": x[b].reshape(HW, CH)[HW // 2:],
            "krw": krw,
            "krh": krh,
        }
        for b in range(8)
    ]
    res = run_bass_kernel_spmd(_NC, in_maps, list(range(8)))
    kernel.last_result = res
    out = np.stack([res.results[b]["out"].reshape(32, 32, 256) for b in range(8)], 0)
    return out


if __name__ == "__main__":
    nc = build_nc()
    print("built ok")


# revision 26
# speedup vs baseline: 1.1566x; 1.0266x over previous
"""Trainium2 Bass kernel for nn_AttentionAugmentation (v2).

Attention with 2D relative-position logits. B=8, H=W=32, dk=dv=256, Nh=8.
Sharding: data-parallel over batch (one batch per NeuronCore, 8 cores).

Per-core algorithm (one batch, 8 heads of 1024x1024 attention, dkh=32):
  - inputs loaded as three column-split casting DMAs (q, k, v) so the q
    transposes / rel-logit matmuls start ~3us in, before k/v land.
  - PE warm-up with real matmuls (transposes don't open the HAM clock gate).
  - q/k PE-transposed in 4-head groups -> DVE copy (q scaled) -> partition-
    scatter DMAs into rows 0-31 of the augmented operands qaug/kaug.
  - relative logits folded into the main QK matmul via a 96-row augmented
    contraction: rows 32-63 (WRELT | one-hot of key y2), rows 64-95
    (HRELT | one-hot of key x2). WRELT/HRELT built with one small matmul
    per (y or x, 4-batch) over all heads; psum batched [32,1024] and copied
    with wide 4D-AP copies split across ACT/DVE (the old per-slice scalar
    copies serialized the whole front half of the kernel).
  - S^T = kaug^T @ qaug per 128-key chunk; exp split between ScalarE
    (true exp) and VectorE (Schraudolph: (int16)(A*S + B) bitcast as bf16
    approximates e^S to ~3%; C tuned for zero mean bias so ACT- and
    DVE-exp'd key chunks agree); AV with lhsT=[V | 1] giving attn^T rows
    plus softmax denominators.
  - attn^T PE-transposed back per 128-query chunk, scaled by reciprocal
    denominators into out_sb; per-pair output DMAs overlap the tail.
Matmul operands bf16 (fp32 PSUM accumulation).
"""
import sys

sys.path.insert(0, "/opt/trn_rl_repo")

from contextlib import ExitStack

import numpy as np

import concourse.bass as bass
from concourse import bacc
import concourse.mybir as mybir
from concourse import masks
from concourse.tile import TileContext

HW = 1024
CH = 768
NH = 8
F32 = mybir.dt.float32
BF16 = mybir.dt.bfloat16
I16 = mybir.dt.int16
EXP = mybir.ActivationFunctionType.Exp
MULT = mybir.AluOpType.mult
ADD = mybir.AluOpType.add
QSCALE = float((256 / 8) ** -0.5)
# Schraudolph exp for bf16: (int16)(A*x + B) bits ~= bf16(e^x).
SCH_A = 184.6649652337873      # 2^7 / ln(2)
SCH_B = 16256.0 - 7.0          # 127 * 2^7 + C (C=-7: zero mean ratio bias)
DVE_CHUNKS = (3, 7)            # key chunks exp'd on VectorE (rest ScalarE)


def build_nc():
    nc = bacc.Bacc()
    # input split in two halves: a single [1024, 768] parameter makes the
    # axon-pjrt reshard program's dynamic-slice exceed a 16-bit semaphore
    # field in neuronx-cc (25MB concat across 8 cores), crashing walrus.
    xa_d = nc.declare_dram_parameter("xa", [HW // 2, CH], F32, isOutput=False)
    xb_d = nc.declare_dram_parameter("xb", [HW // 2, CH], F32, isOutput=False)
    krw_d = nc.declare_dram_parameter("krw", [63, 32], F32, isOutput=False)
    krh_d = nc.declare_dram_parameter("krh", [63, 32], F32, isOutput=False)
    out_d = nc.declare_dram_parameter("out", [HW, 256], F32, isOutput=True)

    with ExitStack() as octx:
        tc = octx.enter_context(TileContext(nc))
        sb = octx.enter_context(tc.tile_pool(name="persist", bufs=1))

        x_sb = sb.tile([128, 6144], BF16)       # (c, e): chunk c, channel e
        qaug = sb.tile([96, NH * HW], BF16)     # per head h: cols 1024h + (32x + y)
        kaug = sb.tile([96, NH * HW], BF16)
        v1 = sb.tile([128, NH * 8 * 33], BF16)  # per (h,c): 33 cols = V chunk | ones
        tscr = sb.tile([128, 4 * HW], BF16)     # transpose scratch (kind, group)
        out_sb = sb.tile([128, 8 * 256], F32)   # col 256c + 32h + d
        identb = sb.tile([128, 128], BF16)
        identf = sb.tile([128, 128], F32)
        krw_sb = sb.tile([63, 32], BF16)
        krh_sb = sb.tile([63, 32], BF16)
        krwT = sb.tile([32, 63], BF16)
        krhT = sb.tile([32, 63], BF16)

        # ---- identity first (gates PE warm-up), then input DMAs ----
        masks.make_identity(nc, identb[:])

        # small rel tables first (krT transposes consume them immediately),
        # then full-bandwidth contiguous chunk loads (fp32 -> bf16 SWDGE)
        nc.gpsimd.dma_start(out=krw_sb[:], in_=krw_d[:])
        nc.gpsimd.dma_start(out=krh_sb[:], in_=krh_d[:])
        for c in range(8):
            src_d = xa_d if c < 4 else xb_d
            cc = c % 4
            nc.gpsimd.dma_start(
                out=x_sb[:, 768 * c:768 * c + 768],
                in_=src_d[128 * cc:128 * cc + 128, :],
            )

        # ---- remaining constants (gpsimd queue, after DMA issues) ----
        masks.make_identity(nc, identf[:])
        nc.gpsimd.memset(kaug[32:64, 0:HW], 0.0)
        nc.gpsimd.memset(kaug[64:96, 0:HW], 0.0)
        # one-hot blocks, head-0 only: rows 32-63: [y2(k)==j], rows 64-95:
        # [x2(k)==j]; key col = 32*x2 + y2. DMA-replicated to heads 1-7.
        nc.gpsimd.affine_select(
            out=kaug[32:64, 0:HW].rearrange("p (x y) -> p x y", x=32, y=32),
            in_=kaug[32:64, 0:HW].rearrange("p (x y) -> p x y", x=32, y=32),
            compare_op=mybir.AluOpType.not_equal,
            fill=1.0,
            base=0,
            pattern=[[0, 32], [-1, 32]],
            channel_multiplier=1,
        )
        nc.gpsimd.affine_select(
            out=kaug[64:96, 0:HW].rearrange("p (x y) -> p x y", x=32, y=32),
            in_=kaug[64:96, 0:HW].rearrange("p (x y) -> p x y", x=32, y=32),
            compare_op=mybir.AluOpType.not_equal,
            fill=1.0,
            base=0,
            pattern=[[-1, 32], [0, 32]],
            channel_multiplier=1,
        )
        # one-hot replication on the gpsimd DMA queue (keep sync/scalar free
        # for the latency-critical q/k partition-scatters)
        for base in (32, 64):
            n = HW
            while n < NH * HW:
                rep = min(n, NH * HW - n)
                nc.gpsimd.dma_start(
                    out=kaug[base:base + 32, n:n + rep],
                    in_=kaug[base:base + 32, 0:rep],
                )
                n += rep
        nc.gpsimd.memset(v1[:], 1.0)

        qa4 = qaug[0:32, :].rearrange("p (h x y) -> p h x y", h=8, x=32, y=32)
        wdst = qaug[32:64, :].rearrange("p (h x y) -> p h x y", h=8, x=32, y=32)
        hdst = qaug[64:96, :].rearrange("p (h x y) -> p h x y", h=8, x=32, y=32)

        # ================= Phase A =================
        with ExitStack() as actx:
            psA = actx.enter_context(tc.tile_pool(name="psA", bufs=2, space="PSUM"))

            # filler matmuls woven between transposes / rel batches keep the
            # HAM clock gate open (transposes and sparse small matmuls alone
            # leave the PE at 1.2 GHz). Fillers borrow idle pool slots.
            fillA = psA.tile([32, 1024], F32, tag="rel", bufs=2, name="fillA")

            def filler(ft, n=2):
                for _ in range(n):
                    nc.tensor.matmul(
                        out=ft[0:32, 0:128], lhsT=identb[0:32, 0:32],
                        rhs=identb[0:32, :], start=True, stop=True,
                    )

            # qT/kT psum tiles (kr transposes borrow two slots first)
            pts = {}
            for kind in range(2):
                for g in range(2):
                    pts[(kind, g)] = psA.tile([128, HW], BF16, tag="tps",
                                              bufs=4, name=f"pt{kind}{g}")

            # key_rel transposes: krw [63,32] -> krwT [32,63]. Borrow the tail
            # (chunk-7 region) of two pt slots — those transposes run last,
            # long after the krT copies drain.
            for srct, dst, pt in ((krw_sb, krwT, pts[(0, 0)]),
                                  (krh_sb, krhT, pts[(0, 1)])):
                nc.tensor.transpose(
                    out=pt[0:32, 896:959], in_=srct[:],
                    identity=identb[0:63, 0:63]
                )
                nc.vector.tensor_copy(out=dst[:], in_=pt[0:32, 896:959])

            # qT/kT: PE-transpose 4-head groups per input chunk as it lands,
            # fillers between, then DVE copies + partition-scatter DMAs.
            for c in range(8):
                for kind in range(2):
                    for g in range(2):
                        nc.tensor.transpose(
                            out=pts[(kind, g)][:, 128 * c:128 * c + 128],
                            in_=x_sb[:, 768 * c + 256 * kind + 128 * g:
                                     768 * c + 256 * kind + 128 * g + 128],
                            identity=identb[:, 0:128],
                        )
                filler(fillA, 2)
            for kind in range(2):
                dstt = qaug if kind == 0 else kaug
                for g in range(2):
                    pt = pts[(kind, g)]
                    scr = tscr[:, HW * (2 * kind + g):HW * (2 * kind + g) + HW]
                    if kind == 0:
                        nc.vector.tensor_scalar_mul(out=scr, in0=pt[:], scalar1=QSCALE)
                    else:
                        nc.vector.tensor_copy(out=scr, in_=pt[:])
                    # partition-scatter DMAs split across two issue queues
                    for hh in range(4):
                        h = 4 * g + hh
                        eng = nc.sync if hh % 2 == 0 else nc.scalar
                        eng.dma_start(
                            out=dstt[0:32, HW * h:HW * h + HW],
                            in_=tscr[32 * hh:32 * hh + 32,
                                     HW * (2 * kind + g):HW * (2 * kind + g) + HW],
                        )

            # rel logits, batched: per 4 x-positions (H) / y-positions (W)
            # one [32, 1024] psum + one wide 4D-AP copy. Fillers (into a
            # retired pt slot) keep the PE warm through the copy waits.
            fillB = psA.tile([128, HW], BF16, tag="tps", bufs=4, name="fillB")
            fillBf = fillB[:].bitcast(F32)
            for t in range(8):
                pR = psA.tile([32, 1024], F32, tag="rel", bufs=2)
                pv = pR[:].rearrange("p (i h y) -> p i h y", i=4, h=8, y=32)
                for xi in range(4):
                    x = 4 * t + xi
                    nc.tensor.matmul(
                        out=pv[:, xi, :, :],
                        lhsT=krhT[:, 31 - x:63 - x],
                        rhs=qa4[:, :, x, :],
                        start=True, stop=True,
                    )
                dst = hdst[:, :, 4 * t:4 * t + 4, :]
                src = pv[:].rearrange("p i h y -> p h i y")
                if t % 2 == 0:
                    nc.vector.tensor_copy(out=dst, in_=src)
                else:
                    nc.scalar.copy(out=dst, in_=src)
                filler(fillBf, 2)
            for t in range(8):
                pR = psA.tile([32, 1024], F32, tag="rel", bufs=2)
                pw = pR[:].rearrange("p (i h x) -> p i h x", i=4, h=8, x=32)
                for yi in range(4):
                    y = 4 * t + yi
                    nc.tensor.matmul(
                        out=pw[:, yi, :, :],
                        lhsT=krwT[:, 31 - y:63 - y],
                        rhs=qa4[:, :, :, y],
                        start=True, stop=True,
                    )
                dst = wdst[:, :, :, 4 * t:4 * t + 4]
                src = pw[:].rearrange("p i h x -> p h x i")
                if t % 2 == 0:
                    nc.vector.tensor_copy(out=dst, in_=src)
                else:
                    nc.scalar.copy(out=dst, in_=src)
                filler(fillBf, 2)

            # V chunks into v1 via SBUF->SBUF DMAs (64B runs)
            v1v = v1[:].rearrange("p (h c e) -> p c h e", h=8, c=8, e=33)
            for c in range(8):
                nc.sync.dma_start(
                    out=v1v[:, c, :, 0:32],
                    in_=x_sb[:, 768 * c + 512:768 * c + 768]
                        .rearrange("p (h e) -> p h e", h=8),
                )

        # ================= Phase B: attention =================
        with ExitStack() as bctx:
            psB = bctx.enter_context(tc.tile_pool(name="psB", bufs=1, space="PSUM"))
            sbW = bctx.enter_context(tc.tile_pool(name="sbW", bufs=3))
            sbA = bctx.enter_context(tc.tile_pool(name="sbA", bufs=2))
            sbR = bctx.enter_context(tc.tile_pool(name="sbR", bufs=3))

            def pair_chunks(hp, mid_cb=None):
                """QK -> exp -> AV for the two heads of pair hp; returns the
                att psum -> SBUF staging tiles (copies emitted here so the
                att psum slots free up for the next pair ASAP)."""
                att = [psB.tile([97, 512], F32, tag=f"att{e}", bufs=1,
                                name=f"att{e}")
                       for e in range(2)]
                for hh in range(2):
                    if hh == 1 and mid_cb is not None:
                        mid_cb()
                    h = 2 * hp + hh
                    pb = 64 * hh
                    for c in range(8):
                        s_ps = psB.tile([128, HW], F32, tag="s", bufs=2)
                        for e in range(2):
                            nc.tensor.matmul(
                                out=s_ps[:, 512 * e:512 * e + 512],
                                lhsT=kaug[:, HW * h + 128 * c:HW * h + 128 * c + 128],
                                rhs=qaug[:, HW * h + 512 * e:HW * h + 512 * e + 512],
                                start=True, stop=True,
                            )
                        wexp = sbW.tile([128, HW], BF16, tag="wexp")
                        if c in DVE_CHUNKS:
                            nc.vector.tensor_scalar(
                                out=wexp[:].bitcast(I16),
                                in0=s_ps[:],
                                scalar1=SCH_A, scalar2=SCH_B,
                                op0=MULT, op1=ADD,
                            )
                        else:
                            nc.scalar.activation(out=wexp[:], in_=s_ps[:], func=EXP)
                        for e in range(2):
                            nc.tensor.matmul(
                                out=att[e][pb:pb + 33, :],
                                lhsT=v1[:, 264 * h + 33 * c:264 * h + 33 * c + 33],
                                rhs=wexp[:, 512 * e:512 * e + 512],
                                start=(c == 0), stop=(c == 7),
                            )
                att_sb = []
                for e in range(2):
                    asb = sbA.tile([97, 512], F32, tag="attsb", bufs=4,
                                   name=f"attsb{e}")
                    nc.vector.tensor_copy(out=asb[:], in_=att[e][:])
                    att_sb.append(asb)
                return att_sb

            def pair_output(hp, att_sb):
                """Transpose/scale/DMA tail for pair hp (emitted one pair
                late so it doesn't block the next pair's matmuls)."""
                for e in range(2):
                    for ci in range(4):
                        c = 4 * e + ci
                        ot = psB.tile([128, 97], F32, tag="ot", bufs=2)
                        nc.tensor.transpose(
                            out=ot[:],
                            in_=att_sb[e][:, 128 * ci:128 * ci + 128],
                            identity=identf[0:97, 0:97],
                        )
                        rc = sbR.tile([128, 2], F32, tag="rc")
                        nc.vector.reciprocal(out=rc[:], in_=ot[:, 32:97:64])
                        for hh in range(2):
                            nc.vector.tensor_scalar_mul(
                                out=out_sb[:, 256 * c + 64 * hp + 32 * hh:
                                           256 * c + 64 * hp + 32 * hh + 32],
                                in0=ot[:, 64 * hh:64 * hh + 32],
                                scalar1=rc[:, hh:hh + 1],
                            )
                # per-pair output DMA (cols 64hp..64hp+63 of each 256-block)
                nc.sync.dma_start(
                    out=out_d[:].rearrange("(c p) d -> p c d", p=128)
                        [:, :, 64 * hp:64 * hp + 64],
                    in_=out_sb[:].rearrange("p (c d) -> p c d", c=8)
                        [:, :, 64 * hp:64 * hp + 64],
                )

            pending = None
            for hp in range(NH // 2):
                prev = pending
                att_sb = pair_chunks(
                    hp,
                    mid_cb=(lambda p=prev: pair_output(*p)) if prev else None,
                )
                pending = (hp, att_sb)
            pair_output(*pending)
    if not nc.is_finalized():
        nc.finalize()
    return nc


_NC = None


def _ensure_axon_hooks_module():
    """bass_utils imports antenv.axon_hooks unconditionally when trace=True;
    this image's antenv lacks it. Provide a stub so tracing degrades to
    no-trace instead of crashing (a real hook can be set by a profiler)."""
    import types

    if "antenv.axon_hooks" in sys.modules:
        return
    try:
        import antenv.axon_hooks  # noqa: F401
        return
    except ImportError:
        pass
    try:
        import antenv
    except ImportError:
        return
    m = types.ModuleType("antenv.axon_hooks")
    m._hook = None
    m.get_axon_ntff_profile_hook = lambda: m._hook
    m.set_axon_ntff_profile_hook = lambda h: setattr(m, "_hook", h)
    sys.modules["antenv.axon_hooks"] = m
    antenv.axon_hooks = m


def kernel(**inputs):
    global _NC
    x = np.ascontiguousarray(np.asarray(inputs["inputs"], dtype=np.float32))
    krw = np.ascontiguousarray(np.asarray(inputs["key_rel_w"], dtype=np.float32))
    krh = np.ascontiguousarray(np.asarray(inputs["key_rel_h"], dtype=np.float32))
    assert x.shape == (8, 32, 32, 768), x.shape
    assert int(inputs["dk"]) == 256 and int(inputs["dv"]) == 256
    assert int(inputs["Nh"]) == 8

    if _NC is None:
        _NC = build_nc()
    _ensure_axon_hooks_module()
    from concourse.bass_utils import run_bass_kernel_spmd

    in_maps = [
        {
            "xa": x[b].reshape(HW, CH)[:HW // 2],
            "xb": x[b].reshape(HW, CH)[HW // 2:],
            "krw": krw,
            "krh": krh,
        }
        for b in range(8)
    ]
    res = run_bass_kernel_spmd(_NC, in_maps, list(range(8)))
    kernel.last_result = res
    out = np.stack([res.results[b]["out"].reshape(32, 32, 256) for b in range(8)], 0)
    return out


if __name__ == "__main__":
    nc = build_nc()
    print("built ok")


# revision 31
# speedup vs baseline: 1.1700x; 1.0116x over previous
"""Trainium2 Bass kernel for nn_AttentionAugmentation (v2).

Attention with 2D relative-position logits. B=8, H=W=32, dk=dv=256, Nh=8.
Sharding: data-parallel over batch (one batch per NeuronCore, 8 cores).

Per-core algorithm (one batch, 8 heads of 1024x1024 attention, dkh=32):
  - inputs loaded as three column-split casting DMAs (q, k, v) so the q
    transposes / rel-logit matmuls start ~3us in, before k/v land.
  - PE warm-up with real matmuls (transposes don't open the HAM clock gate).
  - q/k PE-transposed in 4-head groups -> DVE copy (q scaled) -> partition-
    scatter DMAs into rows 0-31 of the augmented operands qaug/kaug.
  - relative logits folded into the main QK matmul via a 96-row augmented
    contraction: rows 32-63 (WRELT | one-hot of key y2), rows 64-95
    (HRELT | one-hot of key x2). WRELT/HRELT built with one small matmul
    per (y or x, 4-batch) over all heads; psum batched [32,1024] and copied
    with wide 4D-AP copies split across ACT/DVE (the old per-slice scalar
    copies serialized the whole front half of the kernel).
  - S^T = kaug^T @ qaug per 128-key chunk; exp split between ScalarE
    (true exp) and VectorE (Schraudolph: (int16)(A*S + B) bitcast as bf16
    approximates e^S to ~3%; C tuned for zero mean bias so ACT- and
    DVE-exp'd key chunks agree); AV with lhsT=[V | 1] giving attn^T rows
    plus softmax denominators.
  - attn^T PE-transposed back per 128-query chunk, scaled by reciprocal
    denominators into out_sb; per-pair output DMAs overlap the tail.
Matmul operands bf16 (fp32 PSUM accumulation).
"""
import sys

sys.path.insert(0, "/opt/trn_rl_repo")

from contextlib import ExitStack

import numpy as np

import concourse.bass as bass
from concourse import bacc
import concourse.mybir as mybir
from concourse import masks
from concourse.tile import TileContext

HW = 1024
CH = 768
NH = 8
F32 = mybir.dt.float32
BF16 = mybir.dt.bfloat16
I16 = mybir.dt.int16
EXP = mybir.ActivationFunctionType.Exp
MULT = mybir.AluOpType.mult
ADD = mybir.AluOpType.add
QSCALE = float((256 / 8) ** -0.5)
# Schraudolph exp for bf16: (int16)(A*x + B) bits ~= bf16(e^x).
SCH_A = 184.6649652337873      # 2^7 / ln(2)
SCH_B = 16256.0 - 7.0          # 127 * 2^7 + C (C=-7: zero mean ratio bias)
DVE_CHUNKS = (3, 7)            # key chunks exp'd on VectorE (rest ScalarE)


def build_nc():
    nc = bacc.Bacc()
    # input split in two halves: a single [1024, 768] parameter makes the
    # axon-pjrt reshard program's dynamic-slice exceed a 16-bit semaphore
    # field in neuronx-cc (25MB concat across 8 cores), crashing walrus.
    xa_d = nc.declare_dram_parameter("xa", [HW // 2, CH], F32, isOutput=False)
    xb_d = nc.declare_dram_parameter("xb", [HW // 2, CH], F32, isOutput=False)
    krw_d = nc.declare_dram_parameter("krw", [63, 32], F32, isOutput=False)
    krh_d = nc.declare_dram_parameter("krh", [63, 32], F32, isOutput=False)
    out_d = nc.declare_dram_parameter("out", [HW, 256], F32, isOutput=True)

    with ExitStack() as octx:
        tc = octx.enter_context(TileContext(nc))
        sb = octx.enter_context(tc.tile_pool(name="persist", bufs=1))

        x_sb = sb.tile([128, 6144], BF16)       # (c, e): chunk c, channel e
        qaug = sb.tile([96, NH * HW], BF16)     # per head h: cols 1024h + (32x + y)
        kaug = sb.tile([96, NH * HW], BF16)
        v1 = sb.tile([128, NH * 8 * 33], BF16)  # per (h,c): 33 cols = V chunk | ones
        tscr = sb.tile([128, 4 * HW], BF16)     # transpose scratch (kind, group)
        out_sb = sb.tile([128, 8 * 256], F32)   # col 256c + 32h + d
        identb = sb.tile([128, 128], BF16)
        identf = sb.tile([128, 128], F32)
        krw_sb = sb.tile([63, 32], BF16)
        krh_sb = sb.tile([63, 32], BF16)
        krwT = sb.tile([32, 63], BF16)
        krhT = sb.tile([32, 63], BF16)

        # ---- identity first (gates PE warm-up), then input DMAs ----
        masks.make_identity(nc, identb[:])

        # small rel tables first (krT transposes consume them immediately),
        # then full-bandwidth contiguous chunk loads. Split across two DMA
        # queues: even chunks SWDGE-cast on gpsimd, odd chunks fp32 on sync
        # (HWDGE can't cast) + DVE bf16 casts.
        nc.gpsimd.dma_start(out=krw_sb[:], in_=krw_d[:])
        nc.gpsimd.dma_start(out=krh_sb[:], in_=krh_d[:])
        x32_sb = sb.tile([128, 4 * 768], F32)
        for c in range(8):
            src_d = xa_d if c < 4 else xb_d
            cc = c % 4
            if c % 2 == 0:
                nc.gpsimd.dma_start(
                    out=x_sb[:, 768 * c:768 * c + 768],
                    in_=src_d[128 * cc:128 * cc + 128, :],
                )
            else:
                nc.sync.dma_start(
                    out=x32_sb[:, 768 * (c // 2):768 * (c // 2) + 768],
                    in_=src_d[128 * cc:128 * cc + 128, :],
                )
        for c in (1, 3, 5, 7):
            nc.vector.tensor_copy(
                out=x_sb[:, 768 * c:768 * c + 768],
                in_=x32_sb[:, 768 * (c // 2):768 * (c // 2) + 768],
            )

        # ---- remaining constants (gpsimd queue, after DMA issues) ----
        masks.make_identity(nc, identf[:])
        nc.gpsimd.memset(kaug[32:64, 0:HW], 0.0)
        nc.gpsimd.memset(kaug[64:96, 0:HW], 0.0)
        # one-hot blocks, head-0 only: rows 32-63: [y2(k)==j], rows 64-95:
        # [x2(k)==j]; key col = 32*x2 + y2. DMA-replicated to heads 1-7.
        nc.gpsimd.affine_select(
            out=kaug[32:64, 0:HW].rearrange("p (x y) -> p x y", x=32, y=32),
            in_=kaug[32:64, 0:HW].rearrange("p (x y) -> p x y", x=32, y=32),
            compare_op=mybir.AluOpType.not_equal,
            fill=1.0,
            base=0,
            pattern=[[0, 32], [-1, 32]],
            channel_multiplier=1,
        )
        nc.gpsimd.affine_select(
            out=kaug[64:96, 0:HW].rearrange("p (x y) -> p x y", x=32, y=32),
            in_=kaug[64:96, 0:HW].rearrange("p (x y) -> p x y", x=32, y=32),
            compare_op=mybir.AluOpType.not_equal,
            fill=1.0,
            base=0,
            pattern=[[-1, 32], [0, 32]],
            channel_multiplier=1,
        )
        # one-hot replication on the gpsimd DMA queue (keep sync/scalar free
        # for the latency-critical q/k partition-scatters)
        for base in (32, 64):
            n = HW
            while n < NH * HW:
                rep = min(n, NH * HW - n)
                nc.gpsimd.dma_start(
                    out=kaug[base:base + 32, n:n + rep],
                    in_=kaug[base:base + 32, 0:rep],
                )
                n += rep
        nc.gpsimd.memset(v1[:], 1.0)

        qa4 = qaug[0:32, :].rearrange("p (h x y) -> p h x y", h=8, x=32, y=32)
        wdst = qaug[32:64, :].rearrange("p (h x y) -> p h x y", h=8, x=32, y=32)
        hdst = qaug[64:96, :].rearrange("p (h x y) -> p h x y", h=8, x=32, y=32)

        # ================= Phase A =================
        with ExitStack() as actx:
            psA = actx.enter_context(tc.tile_pool(name="psA", bufs=2, space="PSUM"))

            # filler matmuls woven between transposes / rel batches keep the
            # HAM clock gate open (transposes and sparse small matmuls alone
            # leave the PE at 1.2 GHz). Fillers borrow idle pool slots.
            fillA = psA.tile([32, 1024], F32, tag="rel", bufs=2, name="fillA")

            def filler(ft, n=2):
                for _ in range(n):
                    nc.tensor.matmul(
                        out=ft[0:32, 0:128], lhsT=identb[0:32, 0:32],
                        rhs=identb[0:32, :], start=True, stop=True,
                    )

            # qT/kT psum tiles (kr transposes borrow two slots first)
            pts = {}
            for kind in range(2):
                for g in range(2):
                    pts[(kind, g)] = psA.tile([128, HW], BF16, tag="tps",
                                              bufs=4, name=f"pt{kind}{g}")

            # key_rel transposes: krw [63,32] -> krwT [32,63]. Borrow the tail
            # (chunk-7 region) of two pt slots — those transposes run last,
            # long after the krT copies drain.
            for srct, dst, pt in ((krw_sb, krwT, pts[(0, 0)]),
                                  (krh_sb, krhT, pts[(0, 1)])):
                nc.tensor.transpose(
                    out=pt[0:32, 896:959], in_=srct[:],
                    identity=identb[0:63, 0:63]
                )
                nc.vector.tensor_copy(out=dst[:], in_=pt[0:32, 896:959])

            # qT/kT: PE-transpose 4-head groups per input chunk as it lands,
            # fillers between, then DVE copies + partition-scatter DMAs.
            for c in range(8):
                for kind in range(2):
                    for g in range(2):
                        nc.tensor.transpose(
                            out=pts[(kind, g)][:, 128 * c:128 * c + 128],
                            in_=x_sb[:, 768 * c + 256 * kind + 128 * g:
                                     768 * c + 256 * kind + 128 * g + 128],
                            identity=identb[:, 0:128],
                        )
                filler(fillA, 2)
            for kind in range(2):
                dstt = qaug if kind == 0 else kaug
                for g in range(2):
                    pt = pts[(kind, g)]
                    scr = tscr[:, HW * (2 * kind + g):HW * (2 * kind + g) + HW]
                    if kind == 0:
                        nc.vector.tensor_scalar_mul(out=scr, in0=pt[:], scalar1=QSCALE)
                    else:
                        nc.vector.tensor_copy(out=scr, in_=pt[:])
                    # partition-scatter DMAs split across two issue queues
                    for hh in range(4):
                        h = 4 * g + hh
                        eng = nc.sync if hh % 2 == 0 else nc.scalar
                        eng.dma_start(
                            out=dstt[0:32, HW * h:HW * h + HW],
                            in_=tscr[32 * hh:32 * hh + 32,
                                     HW * (2 * kind + g):HW * (2 * kind + g) + HW],
                        )

            # rel logits, batched: per 4 x-positions (H) / y-positions (W)
            # one [32, 1024] psum + one wide 4D-AP copy. Fillers (into a
            # retired pt slot) keep the PE warm through the copy waits.
            fillB = psA.tile([128, HW], BF16, tag="tps", bufs=4, name="fillB")
            fillBf = fillB[:].bitcast(F32)
            for t in range(8):
                pR = psA.tile([32, 1024], F32, tag="rel", bufs=2)
                pv = pR[:].rearrange("p (i h y) -> p i h y", i=4, h=8, y=32)
                for xi in range(4):
                    x = 4 * t + xi
                    nc.tensor.matmul(
                        out=pv[:, xi, :, :],
                        lhsT=krhT[:, 31 - x:63 - x],
                        rhs=qa4[:, :, x, :],
                        start=True, stop=True,
                    )
                # split each batch copy across both engines (heads 0-3 DVE,
                # 4-7 ACT) so the copy chain doesn't serialize the batches
                dst = hdst[:, :, 4 * t:4 * t + 4, :]
                src = pv[:].rearrange("p i h y -> p h i y")
                nc.vector.tensor_copy(out=dst[:, 0:4], in_=src[:, 0:4])
                nc.scalar.copy(out=dst[:, 4:8], in_=src[:, 4:8])
                filler(fillBf, 2)
            for t in range(8):
                pR = psA.tile([32, 1024], F32, tag="rel", bufs=2)
                pw = pR[:].rearrange("p (i h x) -> p i h x", i=4, h=8, x=32)
                for yi in range(4):
                    y = 4 * t + yi
                    nc.tensor.matmul(
                        out=pw[:, yi, :, :],
                        lhsT=krwT[:, 31 - y:63 - y],
                        rhs=qa4[:, :, :, y],
                        start=True, stop=True,
                    )
                dst = wdst[:, :, :, 4 * t:4 * t + 4]
                src = pw[:].rearrange("p i h x -> p h x i")
                nc.vector.tensor_copy(out=dst[:, 0:6], in_=src[:, 0:6])
                nc.scalar.copy(out=dst[:, 6:8], in_=src[:, 6:8])
                filler(fillBf, 2)

            # V chunks into v1 via SBUF->SBUF DMAs (64B runs)
            v1v = v1[:].rearrange("p (h c e) -> p c h e", h=8, c=8, e=33)
            for c in range(8):
                nc.sync.dma_start(
                    out=v1v[:, c, :, 0:32],
                    in_=x_sb[:, 768 * c + 512:768 * c + 768]
                        .rearrange("p (h e) -> p h e", h=8),
                )

        # ================= Phase B: attention =================
        with ExitStack() as bctx:
            psB = bctx.enter_context(tc.tile_pool(name="psB", bufs=1, space="PSUM"))
            sbW = bctx.enter_context(tc.tile_pool(name="sbW", bufs=3))
            sbA = bctx.enter_context(tc.tile_pool(name="sbA", bufs=2))
            sbR = bctx.enter_context(tc.tile_pool(name="sbR", bufs=3))

            def pair_chunks(hp, out_cb=None):
                """QK -> exp -> AV for the two heads of pair hp; returns the
                att psum -> SBUF staging tiles (copies emitted here so the
                att psum slots free up for the next pair ASAP). out_cb(i)
                (i=0..15) interleaves the previous pair's output tail one
                transpose per chunk so it never blocks the PE FIFO."""
                att = [psB.tile([97, 512], F32, tag=f"att{e}", bufs=1,
                                name=f"att{e}")
                       for e in range(2)]
                for hh in range(2):
                    h = 2 * hp + hh
                    pb = 64 * hh
                    for c in range(8):
                        if out_cb is not None:
                            out_cb(8 * hh + c)
                        s_ps = psB.tile([128, HW], F32, tag="s", bufs=2)
                        for e in range(2):
                            nc.tensor.matmul(
                                out=s_ps[:, 512 * e:512 * e + 512],
                                lhsT=kaug[:, HW * h + 128 * c:HW * h + 128 * c + 128],
                                rhs=qaug[:, HW * h + 512 * e:HW * h + 512 * e + 512],
                                start=True, stop=True,
                            )
                        wexp = sbW.tile([128, HW], BF16, tag="wexp")
                        if c in DVE_CHUNKS:
                            nc.vector.tensor_scalar(
                                out=wexp[:].bitcast(I16),
                                in0=s_ps[:],
                                scalar1=SCH_A, scalar2=SCH_B,
                                op0=MULT, op1=ADD,
                            )
                        else:
                            nc.scalar.activation(out=wexp[:], in_=s_ps[:], func=EXP)
                        for e in range(2):
                            nc.tensor.matmul(
                                out=att[e][pb:pb + 33, :],
                                lhsT=v1[:, 264 * h + 33 * c:264 * h + 33 * c + 33],
                                rhs=wexp[:, 512 * e:512 * e + 512],
                                start=(c == 0), stop=(c == 7),
                            )
                att_sb = []
                for e in range(2):
                    asb = sbA.tile([97, 512], F32, tag="attsb", bufs=4,
                                   name=f"attsb{e}")
                    nc.vector.tensor_copy(out=asb[:], in_=att[e][:])
                    att_sb.append(asb)
                return att_sb

            def pair_output_step(hp, att_sb, i):
                """One transpose + scale step (i=0..7) of pair hp's tail."""
                e, ci = i // 4, i % 4
                c = 4 * e + ci
                ot = psB.tile([128, 97], F32, tag="ot", bufs=2)
                nc.tensor.transpose(
                    out=ot[:],
                    in_=att_sb[e][:, 128 * ci:128 * ci + 128],
                    identity=identf[0:97, 0:97],
                )
                rc = sbR.tile([128, 2], F32, tag="rc")
                nc.vector.reciprocal(out=rc[:], in_=ot[:, 32:97:64])
                for hh in range(2):
                    nc.vector.tensor_scalar_mul(
                        out=out_sb[:, 256 * c + 64 * hp + 32 * hh:
                                   256 * c + 64 * hp + 32 * hh + 32],
                        in0=ot[:, 64 * hh:64 * hh + 32],
                        scalar1=rc[:, hh:hh + 1],
                    )
                if i == 7:
                    # per-pair output DMA (cols 64hp..+63 of each 256-block)
                    nc.sync.dma_start(
                        out=out_d[:].rearrange("(c p) d -> p c d", p=128)
                            [:, :, 64 * hp:64 * hp + 64],
                        in_=out_sb[:].rearrange("p (c d) -> p c d", c=8)
                            [:, :, 64 * hp:64 * hp + 64],
                    )

            pending = None
            for hp in range(NH // 2):
                prev = pending

                def out_cb(i, p=prev):
                    if p is not None and i % 2 == 0:
                        pair_output_step(p[0], p[1], i // 2)

                att_sb = pair_chunks(hp, out_cb=out_cb if prev else None)
                pending = (hp, att_sb)
            for i in range(8):
                pair_output_step(pending[0], pending[1], i)
    if not nc.is_finalized():
        nc.finalize()
    return nc


_NC = None


def _ensure_axon_hooks_module():
    """bass_utils imports antenv.axon_hooks unconditionally when trace=True;
    this image's antenv lacks it. Provide a stub so tracing degrades to
    no-trace instead of crashing (a real hook can be set by a profiler)."""
    import types

    if "antenv.axon_hooks" in sys.modules:
        return
    try:
        import antenv.axon_hooks  # noqa: F401
        return
    except ImportError:
        pass
    try:
        import antenv
    except ImportError:
        return
    m = types.ModuleType("antenv.axon_hooks")
    m._hook = None
    m.get_axon_ntff_profile_hook = lambda: m._hook
    m.set_axon_ntff_profile_hook = lambda h: setattr(m, "_hook", h)
    sys.modules["antenv.axon_hooks"] = m
    antenv.axon_hooks = m


def kernel(**inputs):
    global _NC
    x = np.ascontiguousarray(np.asarray(inputs["inputs"], dtype=np.float32))
    krw = np.ascontiguousarray(np.asarray(inputs["key_rel_w"], dtype=np.float32))
    krh = np.ascontiguousarray(np.asarray(inputs["key_rel_h"], dtype=np.float32))
    assert x.shape == (8, 32, 32, 768), x.shape
    assert int(inputs["dk"]) == 256 and int(inputs["dv"]) == 256
    assert int(inputs["Nh"]) == 8

    if _NC is None:
        _NC = build_nc()
    _ensure_axon_hooks_module()
    from concourse.bass_utils import run_bass_kernel_spmd

    in_maps = [
        {
            "xa": x[b].reshape(HW, CH)[:HW // 2],
            "xb": x[b].reshape(HW, CH)[HW // 2:],
            "krw": krw,
            "krh": krh,
        }
        for b in range(8)
    ]
    res = run_bass_kernel_spmd(_NC, in_maps, list(range(8)))
    kernel.last_result = res
    out = np.stack([res.results[b]["out"].reshape(32, 32, 256) for b in range(8)], 0)
    return out


if __name__ == "__main__":
    nc = build_nc()
    print("built ok")
